# revision 5
# baseline (speedup 1.0000x reference)
"""HeteroGAT layer Trainium kernel: 8-core dst-sharded edge-parallel.

Strategy: sort each edge set by destination on the host, shard destinations
across the 8 cores (each core owns a contiguous dst range -> no collectives).
On device: phase A computes projected embeddings z = h @ W.T + b and per-node
attention score tables into internal DRAM; phase B processes 128-dst tiles,
building per-128-edge-block one-hot selection matrices (is_equal vs iota) and
doing the two segment sums (denominator, weighted) as PE matmuls into PSUM.
Residual z is recomputed into PSUM per tile; LN + ELU fused on the way out.
"""

import sys
import types
import numpy as np

# -- inject missing antenv.axon_hooks so NTFF tracing works under axon -------
if 'antenv.axon_hooks' not in sys.modules:
    _m = types.ModuleType('antenv.axon_hooks')
    _hh = [None]
    _m.set_axon_ntff_profile_hook = lambda h: _hh.__setitem__(0, h)
    _m.get_axon_ntff_profile_hook = lambda: _hh[0]
    sys.modules['antenv.axon_hooks'] = _m
    try:
        import antenv
        antenv.axon_hooks = _m
        from trn_agent_boot.trn_boot import _ntff_profile_via_ctypes
        _m.set_axon_ntff_profile_hook(
            _ntff_profile_via_ctypes('/opt/axon/libaxon_pjrt.so'))
    except Exception:
        pass

import concourse.bass as bass
import concourse.bacc as bacc
import concourse.mybir as mybir
import concourse.tile as tile
from concourse import bass_utils

F32 = mybir.dt.float32
I32 = mybir.dt.int32
AF = mybir.ActivationFunctionType
OP = mybir.AluOpType

P = 128
NC = 8
N_OP, N_MAC = 50000, 2000
IN_OP, IN_MAC, OUT, HEADS, DK = 64, 32, 128, 4, 32
EPS, LN_EPS = 1e-6, 1e-5
N_OP_PAD = 50176          # 392 * 128
N_MAC_PAD = 2048          # 16 * 128
ND_OP = N_OP // NC        # 6250 dst rows per core
ND_MAC = N_MAC // NC      # 250
T_OP = (ND_OP + P - 1) // P    # 49 tiles (last has 106 rows)
T_MAC = (ND_MAC + P - 1) // P  # 2 tiles (last has 122 rows)

_last_results = [None]


def _proj(att):
    """att [H, 2*DK+1] -> (P_src [OUT,H], P_dst [OUT,H], attc [H])"""
    ps = np.zeros((OUT, HEADS), np.float32)
    pd = np.zeros((OUT, HEADS), np.float32)
    for h in range(HEADS):
        ps[h * DK:(h + 1) * DK, h] = att[h, :DK]
        pd[h * DK:(h + 1) * DK, h] = att[h, DK:2 * DK]
    return ps, pd, np.ascontiguousarray(att[:, 2 * DK]).astype(np.float32)


def _prep_edges(src, dst, feat, nd_core, n_tiles):
    """Sort by dst, shard dst ranges across cores, tile into 128-dst tiles,
    pad each tile's edge list to a block count common across cores.
    Returns per-tile common block counts + per-core packed arrays
    [128, nblk] layouts flattened: value(p,b) = edge b*128+p."""
    order = np.argsort(dst, kind='stable')
    src, dst, feat = src[order], dst[order], feat[order]
    core_lists = []
    for c in range(NC):
        lo, hi = c * nd_core, (c + 1) * nd_core
        a = np.searchsorted(dst, lo)
        b = np.searchsorted(dst, hi)
        s_c, d_c, f_c = src[a:b], dst[a:b] - lo, feat[a:b]
        tiles = []
        for t in range(n_tiles):
            ta = np.searchsorted(d_c, t * P)
            tb = np.searchsorted(d_c, (t + 1) * P)
            tiles.append((s_c[ta:tb], d_c[ta:tb] - t * P, f_c[ta:tb]))
        core_lists.append(tiles)
    nblk = [max(1, max((len(core_lists[c][t][0]) + P - 1) // P
                       for c in range(NC))) for t in range(n_tiles)]
    packed = []   # per core: dict of arrays
    for c in range(NC):
        lo = c * nd_core
        zs, ds, dr, ft = [], [], [], []
        for t in range(n_tiles):
            s_t, drel_t, f_t = core_lists[c][t]
            ne = nblk[t] * P
            pad = ne - len(s_t)
            s_p = np.concatenate([s_t, np.zeros(pad, s_t.dtype)])
            dd = core_lists[c][t][0]  # unused
            drel_p = np.concatenate([drel_t.astype(np.float32),
                                     np.full(pad, 999.0, np.float32)])
            dglob_p = np.concatenate([drel_t + t * P + lo,  # global dst id
                                      np.zeros(pad, drel_t.dtype)])
            f_p = np.concatenate([f_t.astype(np.float32),
                                  np.zeros(pad, np.float32)])
            # [P, nblk] with (p, b) = edge b*128+p
            zs.append(s_p.reshape(nblk[t], P).T)
            ds.append(dglob_p.reshape(nblk[t], P).T)
            dr.append(drel_p.reshape(nblk[t], P).T)
            ft.append(f_p.reshape(nblk[t], P).T)
        packed.append({
            'zidx': np.concatenate([x.reshape(P, -1) for x in zs], 1).astype(np.int32),
            'dloc': np.concatenate([x.reshape(P, -1) for x in ds], 1).astype(np.int32),
            'drel': np.concatenate([x.reshape(P, -1) for x in dr], 1).astype(np.float32),
            'feat': np.concatenate([x.reshape(P, -1) for x in ft], 1).astype(np.float32),
        })
    return nblk, packed


def build_kernel(nblk_seq, nblk_mo, nblk_om):
    nc = bacc.Bacc()
    NB_SEQ, NB_MO, NB_OM = sum(nblk_seq), sum(nblk_mo), sum(nblk_om)

    hT_op = nc.dram_tensor("hT_op", (IN_OP + 1, N_OP_PAD), F32, kind="ExternalInput")
    hT_mac = nc.dram_tensor("hT_mac", (IN_MAC + 1, N_MAC_PAD), F32, kind="ExternalInput")
    hT_own_op = nc.dram_tensor("hT_own_op", (IN_OP + 1, T_OP * P), F32, kind="ExternalInput")
    hT_own_mac = nc.dram_tensor("hT_own_mac", (IN_MAC + 1, T_MAC * P), F32, kind="ExternalInput")
    Wc_op = nc.dram_tensor("Wc_op", (IN_OP + 1, 144), F32, kind="ExternalInput")
    Wc_mac = nc.dram_tensor("Wc_mac", (IN_MAC + 1, 144), F32, kind="ExternalInput")
    consts = nc.dram_tensor("consts", (P, P + 12 + 4 * P), F32, kind="ExternalInput")
    edge_in = {}
    for nm, nb in (("seq", NB_SEQ), ("mo", NB_MO), ("om", NB_OM)):
        for role, dt in (("zidx", I32), ("dloc", I32), ("drel", F32), ("feat", F32)):
            edge_in[f"{nm}_{role}"] = nc.dram_tensor(
                f"{nm}_{role}", (P, nb), dt, kind="ExternalInput")
    out_op = nc.dram_tensor("out_op", (ND_OP, OUT), F32, kind="ExternalOutput")
    out_mac = nc.dram_tensor("out_mac", (ND_MAC, OUT), F32, kind="ExternalOutput")

    with tile.TileContext(nc) as tc:
        with (
            tc.tile_pool(name="con", bufs=1) as con,
            tc.tile_pool(name="sba", bufs=3) as sba,
            tc.tile_pool(name="sbe", bufs=2) as sbe,
            tc.tile_pool(name="sbg", bufs=4) as sbg,
            tc.tile_pool(name="sbom", bufs=1) as sbom,
            tc.tile_pool(name="ps", bufs=2, space="PSUM") as ps,
            tc.tile_pool(name="psd", bufs=2, space="PSUM") as psd,
            tc.tile_pool(name="dram", bufs=1, space="DRAM") as dr,
        ):
            # ---------------- constants ----------------
            ct = con.tile([P, P + 12 + 4 * P], F32)
            nc.sync.dma_start(out=ct[:], in_=consts[:, :])
            iota_t = ct[:, :P]
            attc = {"seq": ct[:, P:P + 4], "mo": ct[:, P + 4:P + 8],
                    "om": ct[:, P + 8:P + 12]}
            g_op = ct[:, P + 12:P + 12 + P]
            b_op = ct[:, P + 12 + P:P + 12 + 2 * P]
            g_mac = ct[:, P + 12 + 2 * P:P + 12 + 3 * P]
            b_mac = ct[:, P + 12 + 3 * P:P + 12 + 4 * P]

            wc_op = con.tile([IN_OP + 1, 144], F32)
            nc.sync.dma_start(out=wc_op[:], in_=Wc_op[:, :])
            wc_mac = con.tile([IN_MAC + 1, 144], F32)
            nc.sync.dma_start(out=wc_mac[:], in_=Wc_mac[:, :])

            # ---------------- phase A: tables ----------------
            z_op = dr.tile([N_OP_PAD, OUT], F32)
            s_op = dr.tile([N_OP_PAD, 16], F32)
            z_mac = dr.tile([N_MAC_PAD, OUT], F32)
            s_mac = dr.tile([N_MAC_PAD, 16], F32)

            def phase_a(hT, k_in, wc, n_pad, z_tab, s_tab):
                ntile = n_pad // P
                for j in range(0, ntile, 4):
                    cols = min(4, ntile - j)
                    ht = sba.tile([k_in, 4 * P], F32, tag="ht")
                    nc.sync.dma_start(out=ht[:, :cols * P],
                                      in_=hT[:, j * P:(j + cols) * P])
                    for q in range(cols):
                        i = j + q
                        pz = ps.tile([P, 144], F32, space="PSUM", tag="pz")
                        nc.tensor.matmul(pz[:], lhsT=ht[:, q * P:(q + 1) * P],
                                         rhs=wc[:], start=True, stop=True)
                        zs = sba.tile([P, 144], F32, tag="zs")
                        if i % 2 == 0:
                            nc.scalar.copy(zs[:], pz[:])
                        else:
                            nc.vector.tensor_copy(zs[:], pz[:])
                        nc.sync.dma_start(out=z_tab[i * P:(i + 1) * P, :],
                                          in_=zs[:, :OUT])
                        nc.sync.dma_start(out=s_tab[i * P:(i + 1) * P, :],
                                          in_=zs[:, OUT:144])

            phase_a(hT_op, IN_OP + 1, wc_op, N_OP_PAD, z_op, s_op)
            phase_a(hT_mac, IN_MAC + 1, wc_mac, N_MAC_PAD, z_mac, s_mac)

            # ---------------- phase B ----------------
            TYPES = {
                "seq": dict(ztab=z_op, s_src=s_op, so=0, s_dst=s_op, do=4),
                "mo":  dict(ztab=z_mac, s_src=s_mac, so=0, s_dst=s_op, do=12),
                "om":  dict(ztab=z_op, s_src=s_op, so=8, s_dst=s_mac, do=4),
            }
            off = {"seq": 0, "mo": 0, "om": 0}

            def do_type(nm, t, nb, p_acc):
                """pass1+pass2 for edge type nm on dst-tile t; accumulate
                weighted sums into p_acc. Returns r tile [P,4]."""
                cfg = TYPES[nm]
                o = off[nm]
                il = sbe.tile([P, 4 * nb], F32, tag="il")
                nc.sync.dma_start(out=il[:, 0 * nb:1 * nb].bitcast(I32),
                                  in_=edge_in[f"{nm}_zidx"][:, o:o + nb])
                nc.sync.dma_start(out=il[:, 1 * nb:2 * nb].bitcast(I32),
                                  in_=edge_in[f"{nm}_dloc"][:, o:o + nb])
                nc.sync.dma_start(out=il[:, 2 * nb:3 * nb],
                                  in_=edge_in[f"{nm}_drel"][:, o:o + nb])
                nc.sync.dma_start(out=il[:, 3 * nb:4 * nb],
                                  in_=edge_in[f"{nm}_feat"][:, o:o + nb])
                zidx = il[:, 0 * nb:1 * nb].bitcast(I32)
                dloc = il[:, 1 * nb:2 * nb].bitcast(I32)
                drel = il[:, 2 * nb:3 * nb]
                feat = il[:, 3 * nb:4 * nb]

                ss = sbe.tile([P, 16 * nb], F32, tag="ssrc")
                sd = sbe.tile([P, 16 * nb], F32, tag="sdst")
                for b in range(nb):
                    nc.gpsimd.indirect_dma_start(
                        out=ss[:, 16 * b:16 * (b + 1)], out_offset=None,
                        in_=cfg["s_src"][:, :],
                        in_offset=bass.IndirectOffsetOnAxis(ap=zidx[:, b:b + 1], axis=0))
                    nc.gpsimd.indirect_dma_start(
                        out=sd[:, 16 * b:16 * (b + 1)], out_offset=None,
                        in_=cfg["s_dst"][:, :],
                        in_offset=bass.IndirectOffsetOnAxis(ap=dloc[:, b:b + 1], axis=0))
                ss3 = ss[:].rearrange("p (n s) -> p n s", n=nb)[:, :, cfg["so"]:cfg["so"] + 4]
                sd3 = sd[:].rearrange("p (n s) -> p n s", n=nb)[:, :, cfg["do"]:cfg["do"] + 4]

                al = sbe.tile([P, 4 * nb], F32, tag="alpha")
                t0 = sbe.tile([P, 4 * nb], F32, tag="sc0")
                # fc = feat * attc
                nc.vector.tensor_tensor(
                    out=t0[:], in0=feat.unsqueeze(2).to_broadcast([P, nb, 4]),
                    in1=attc[nm].unsqueeze(1).to_broadcast([P, nb, 4]), op=OP.mult)
                nc.vector.tensor_tensor(out=al[:], in0=ss3, in1=sd3, op=OP.add)
                nc.vector.tensor_tensor(out=al[:], in0=al[:], in1=t0[:], op=OP.add)
                # leaky relu 0.2 + clip [-20, 20]
                nc.vector.tensor_scalar(out=t0[:], in0=al[:], scalar1=0.2,
                                        scalar2=None, op0=OP.mult)
                nc.vector.tensor_tensor(out=al[:], in0=al[:], in1=t0[:], op=OP.max)
                nc.vector.tensor_scalar(out=al[:], in0=al[:], scalar1=20.0,
                                        scalar2=-20.0, op0=OP.min, op1=OP.max)
                nc.scalar.activation(al[:], al[:], AF.Exp)

                # S matrices (one batched is_equal), denominator matmuls
                pool_s = sbom if nm == "om" else sbe
                S = pool_s.tile([P, nb * P], F32, tag=("Som" if nm == "om" else "S"))
                nc.vector.tensor_tensor(
                    out=S[:],
                    in0=iota_t.unsqueeze(1).to_broadcast([P, nb, P]),
                    in1=drel.unsqueeze(2).to_broadcast([P, nb, P]),
                    op=OP.is_equal)
                pd_ = psd.tile([P, 4], F32, space="PSUM", tag="pd")
                for b in range(nb):
                    nc.tensor.matmul(pd_[:], lhsT=S[:, b * P:(b + 1) * P],
                                     rhs=al[:, 4 * b:4 * (b + 1)],
                                     start=(b == 0), stop=(b == nb - 1))
                r = sbe.tile([P, 4], F32, tag="r")
                nc.vector.tensor_scalar(out=r[:], in0=pd_[:], scalar1=EPS,
                                        scalar2=None, op0=OP.add)
                nc.vector.reciprocal(r[:], r[:])

                # pass 2: gather z rows, scale by alpha, matmul-accumulate
                for b in range(nb):
                    zg = sbg.tile([P, OUT], F32, tag="zg")
                    nc.gpsimd.indirect_dma_start(
                        out=zg[:], out_offset=None, in_=cfg["ztab"][:, :],
                        in_offset=bass.IndirectOffsetOnAxis(ap=zidx[:, b:b + 1], axis=0))
                    zsc = sbg.tile([P, OUT], F32, tag="zsc")
                    nc.vector.tensor_tensor(
                        out=zsc[:], in0=zg[:],
                        in1=al[:, 4 * b:4 * (b + 1)].unsqueeze(2).to_broadcast([P, 4, DK]),
                        op=OP.mult)
                    nc.tensor.matmul(p_acc[:], lhsT=S[:, b * P:(b + 1) * P],
                                     rhs=zsc[:], start=(b == 0), stop=(b == nb - 1))
                off[nm] += nb
                return r

            def post(p_parts, rs, p_z, g_rep, b_rep, out_t, rows, row0):
                """combine attention parts (scaled by r), residual, LN, ELU."""
                res = sbe.tile([P, OUT], F32, tag="res")
                acc = None
                for (p_acc, r) in zip(p_parts, rs):
                    tscale = sbe.tile([P, OUT], F32, tag="tsc")
                    nc.vector.tensor_tensor(
                        out=tscale[:], in0=p_acc[:],
                        in1=r[:].unsqueeze(2).to_broadcast([P, 4, DK]), op=OP.mult)
                    if acc is None:
                        acc = tscale
                    else:
                        nc.vector.tensor_tensor(out=tscale[:], in0=acc[:],
                                                in1=tscale[:], op=OP.add)
                        acc = tscale
                nc.vector.tensor_tensor(out=res[:], in0=acc[:], in1=p_z[:], op=OP.add)
                # layernorm
                mu = sbe.tile([P, 1], F32, tag="mu")
                nc.vector.tensor_reduce(out=mu[:], in_=res[:],
                                        axis=mybir.AxisListType.X, op=OP.add)
                nc.vector.tensor_scalar(out=mu[:], in0=mu[:], scalar1=-1.0 / OUT,
                                        scalar2=None, op0=OP.mult)
                xm = sbe.tile([P, OUT], F32, tag="xm")
                nc.scalar.activation(xm[:], res[:], AF.Identity, bias=mu[:, :1])
                sq = sbe.tile([P, OUT], F32, tag="sq")
                nc.scalar.activation(sq[:], xm[:], AF.Square)
                vs = sbe.tile([P, 1], F32, tag="vs")
                nc.vector.tensor_reduce(out=vs[:], in_=sq[:],
                                        axis=mybir.AxisListType.X, op=OP.add)
                nc.vector.tensor_scalar(out=vs[:], in0=vs[:], scalar1=1.0 / OUT,
                                        scalar2=LN_EPS, op0=OP.mult, op1=OP.add)
                nc.scalar.activation(vs[:], vs[:], AF.Sqrt)
                nc.vector.reciprocal(vs[:], vs[:])
                nc.vector.tensor_scalar(out=xm[:], in0=xm[:], scalar1=vs[:, :1],
                                        scalar2=None, op0=OP.mult)
                nc.vector.tensor_tensor(out=xm[:], in0=xm[:], in1=g_rep, op=OP.mult)
                nc.vector.tensor_tensor(out=xm[:], in0=xm[:], in1=b_rep, op=OP.add)
                # elu(x) = max(x, exp(min(x,0)) - 1)
                el = sbe.tile([P, OUT], F32, tag="el")
                nc.vector.tensor_scalar(out=el[:], in0=xm[:], scalar1=0.0,
                                        scalar2=None, op0=OP.min)
                nc.scalar.activation(el[:], el[:], AF.Exp)
                nc.vector.tensor_scalar(out=el[:], in0=el[:], scalar1=-1.0,
                                        scalar2=None, op0=OP.add)
                nc.vector.tensor_tensor(out=el[:], in0=xm[:], in1=el[:], op=OP.max)
                nc.sync.dma_start(out=out_t[row0:row0 + rows, :], in_=el[:rows, :])

            # op dst tiles
            for t in range(T_OP):
                p_z = ps.tile([P, OUT], F32, space="PSUM", tag="pz")
                hto = sba.tile([IN_OP + 1, P], F32, tag="hto")
                nc.sync.dma_start(out=hto[:], in_=hT_own_op[:, t * P:(t + 1) * P])
                nc.tensor.matmul(p_z[:], lhsT=hto[:], rhs=wc_op[:, :OUT],
                                 start=True, stop=True)
                p_seq = ps.tile([P, OUT], F32, space="PSUM", tag="pseq")
                r_seq = do_type("seq", t, nblk_seq[t], p_seq)
                p_mo = ps.tile([P, OUT], F32, space="PSUM", tag="pmo")
                r_mo = do_type("mo", t, nblk_mo[t], p_mo)
                rows = min(P, ND_OP - t * P)
                post([p_seq, p_mo], [r_seq, r_mo], p_z, g_op, b_op,
                     out_op, rows, t * P)

            # mac dst tiles
            for t in range(T_MAC):
                p_z = ps.tile([P, OUT], F32, space="PSUM", tag="pz")
                htm = sba.tile([IN_MAC + 1, P], F32, tag="htm")
                nc.sync.dma_start(out=htm[:], in_=hT_own_mac[:, t * P:(t + 1) * P])
                nc.tensor.matmul(p_z[:], lhsT=htm[:], rhs=wc_mac[:, :OUT],
                                 start=True, stop=True)
                p_om = ps.tile([P, OUT], F32, space="PSUM", tag="pseq")
                r_om = do_type("om", t, nblk_om[t], p_om)
                rows = min(P, ND_MAC - t * P)
                post([p_om], [r_om], p_z, g_mac, b_mac, out_mac, rows, t * P)

    nc.finalize()
    return nc


def kernel(h_op, h_mac, seq_src, seq_dst, op_mac_src, op_mac_dst,
           mac_op_src, mac_op_dst, feat_seq, feat_op_mac, feat_mac_op,
           W_op_w, W_op_b, W_mac_w, W_mac_b,
           att_seq, att_op_mac, att_mac_op,
           ln_op_g, ln_op_b, ln_mac_g, ln_mac_b):
    tonp = lambda x: np.asarray(x)
    h_op, h_mac = tonp(h_op).astype(np.float32), tonp(h_mac).astype(np.float32)
    idxs = [tonp(x).astype(np.int64) for x in
            (seq_src, seq_dst, op_mac_src, op_mac_dst, mac_op_src, mac_op_dst)]
    seq_src, seq_dst, op_mac_src, op_mac_dst, mac_op_src, mac_op_dst = idxs
    feat_seq = tonp(feat_seq).astype(np.float32).ravel()
    feat_op_mac = tonp(feat_op_mac).astype(np.float32).ravel()
    feat_mac_op = tonp(feat_mac_op).astype(np.float32).ravel()
    W_op_w, W_op_b = tonp(W_op_w).astype(np.float32), tonp(W_op_b).astype(np.float32)
    W_mac_w, W_mac_b = tonp(W_mac_w).astype(np.float32), tonp(W_mac_b).astype(np.float32)
    att_seq, att_op_mac, att_mac_op = (tonp(att_seq).astype(np.float32),
                                       tonp(att_op_mac).astype(np.float32),
                                       tonp(att_mac_op).astype(np.float32))

    # ---- host param prep ----
    ps_seq, pd_seq, ac_seq = _proj(att_seq)
    ps_om, pd_om, ac_om = _proj(att_op_mac)
    ps_mo, pd_mo, ac_mo = _proj(att_mac_op)
    WT_op = W_op_w.T                      # [64,128]
    WT_mac = W_mac_w.T                    # [32,128]
    # s-table cols for op nodes: src_seq, dst_seq, src_om, dst_mo
    Q_op = np.concatenate([WT_op @ ps_seq, WT_op @ pd_seq,
                           WT_op @ ps_om, WT_op @ pd_mo], 1)      # [64,16]
    bq_op = np.concatenate([W_op_b @ ps_seq, W_op_b @ pd_seq,
                            W_op_b @ ps_om, W_op_b @ pd_mo])      # [16]
    # s-table cols for mac nodes: src_mo, dst_om (rest zero)
    Q_mac = np.concatenate([WT_mac @ ps_mo, WT_mac @ pd_om,
                            np.zeros((IN_MAC, 8), np.float32)], 1)
    bq_mac = np.concatenate([W_mac_b @ ps_mo, W_mac_b @ pd_om,
                             np.zeros(8, np.float32)])
    Wc_op = np.zeros((IN_OP + 1, 144), np.float32)
    Wc_op[:IN_OP, :OUT] = WT_op
    Wc_op[IN_OP, :OUT] = W_op_b
    Wc_op[:IN_OP, OUT:] = Q_op
    Wc_op[IN_OP, OUT:] = bq_op
    Wc_mac = np.zeros((IN_MAC + 1, 144), np.float32)
    Wc_mac[:IN_MAC, :OUT] = WT_mac
    Wc_mac[IN_MAC, :OUT] = W_mac_b
    Wc_mac[:IN_MAC, OUT:] = Q_mac
    Wc_mac[IN_MAC, OUT:] = bq_mac

    hT_op = np.zeros((IN_OP + 1, N_OP_PAD), np.float32)
    hT_op[:IN_OP, :N_OP] = h_op.T
    hT_op[IN_OP, :] = 1.0
    hT_mac = np.zeros((IN_MAC + 1, N_MAC_PAD), np.float32)
    hT_mac[:IN_MAC, :N_MAC] = h_mac.T
    hT_mac[IN_MAC, :] = 1.0

    consts_base = np.zeros((P, P + 12 + 4 * P), np.float32)
    consts_base[:, :P] = np.arange(P, dtype=np.float32)[None, :]
    consts_base[:, P:P + 4] = ac_seq[None, :]
    consts_base[:, P + 4:P + 8] = ac_mo[None, :]
    consts_base[:, P + 8:P + 12] = ac_om[None, :]
    consts_base[:, P + 12:P + 12 + P] = tonp(ln_op_g).astype(np.float32)[None, :]
    consts_base[:, P + 12 + P:P + 12 + 2 * P] = tonp(ln_op_b).astype(np.float32)[None, :]
    consts_base[:, P + 12 + 2 * P:P + 12 + 3 * P] = tonp(ln_mac_g).astype(np.float32)[None, :]
    consts_base[:, P + 12 + 3 * P:P + 12 + 4 * P] = tonp(ln_mac_b).astype(np.float32)[None, :]

    nblk_seq, pk_seq = _prep_edges(seq_src, seq_dst, feat_seq, ND_OP, T_OP)
    nblk_mo, pk_mo = _prep_edges(mac_op_src, mac_op_dst, feat_mac_op, ND_OP, T_OP)
    nblk_om, pk_om = _prep_edges(op_mac_src, op_mac_dst, feat_op_mac, ND_MAC, T_MAC)

    in_maps = []
    for c in range(NC):
        m = {"hT_op": hT_op, "hT_mac": hT_mac,
             "Wc_op": Wc_op, "Wc_mac": Wc_mac, "consts": consts_base}
        ho = np.zeros((IN_OP + 1, T_OP * P), np.float32)
        ho[:, :ND_OP] = hT_op[:, c * ND_OP:(c + 1) * ND_OP]
        m["hT_own_op"] = ho
        hm = np.zeros((IN_MAC + 1, T_MAC * P), np.float32)
        hm[:, :ND_MAC] = hT_mac[:, c * ND_MAC:(c + 1) * ND_MAC]
        m["hT_own_mac"] = hm
        for nm, pk in (("seq", pk_seq), ("mo", pk_mo), ("om", pk_om)):
            m[f"{nm}_zidx"] = pk[c]['zidx']
            m[f"{nm}_dloc"] = pk[c]['dloc']
            m[f"{nm}_drel"] = pk[c]['drel']
            m[f"{nm}_feat"] = pk[c]['feat']
        in_maps.append(m)

    nc = build_kernel(nblk_seq, nblk_mo, nblk_om)
    import os
    res = bass_utils.run_bass_kernel_spmd(
        nc, in_maps, core_ids=list(range(NC)),
        trace=bool(os.environ.get("BASS_TRACE")))
    _last_results[0] = res
    res_op = np.concatenate([res.results[c]["out_op"] for c in range(NC)], 0)
    res_mac = np.concatenate([res.results[c]["out_mac"] for c in range(NC)], 0)
    return res_op, res_mac


# revision 6
# speedup vs baseline: 1.0259x; 1.0259x over previous
"""HeteroGAT layer Trainium kernel: 8-core dst-sharded edge-parallel.

Strategy: sort each edge set by destination on the host, shard destinations
across the 8 cores (each core owns a contiguous dst range -> no collectives).
On device: phase A computes projected embeddings z = h @ W.T + b and per-node
attention score tables into internal DRAM; phase B processes 128-dst tiles,
building per-128-edge-block one-hot selection matrices (is_equal vs iota) and
doing the two segment sums (denominator, weighted) as PE matmuls into PSUM.
Residual z is recomputed into PSUM per tile; LN + ELU fused on the way out.
"""

import sys
import types
import numpy as np

# -- inject missing antenv.axon_hooks so NTFF tracing works under axon -------
if 'antenv.axon_hooks' not in sys.modules:
    _m = types.ModuleType('antenv.axon_hooks')
    _hh = [None]
    _m.set_axon_ntff_profile_hook = lambda h: _hh.__setitem__(0, h)
    _m.get_axon_ntff_profile_hook = lambda: _hh[0]
    sys.modules['antenv.axon_hooks'] = _m
    try:
        import antenv
        antenv.axon_hooks = _m
        from trn_agent_boot.trn_boot import _ntff_profile_via_ctypes
        _m.set_axon_ntff_profile_hook(
            _ntff_profile_via_ctypes('/opt/axon/libaxon_pjrt.so'))
    except Exception:
        pass

import concourse.bass as bass
import concourse.bacc as bacc
import concourse.mybir as mybir
import concourse.tile as tile
from concourse import bass_utils

F32 = mybir.dt.float32
BF16 = mybir.dt.bfloat16
I32 = mybir.dt.int32
AF = mybir.ActivationFunctionType
OP = mybir.AluOpType

P = 128
NC = 8
N_OP, N_MAC = 50000, 2000
IN_OP, IN_MAC, OUT, HEADS, DK = 64, 32, 128, 4, 32
EPS, LN_EPS = 1e-6, 1e-5
N_OP_PAD = 50176          # 392 * 128
N_MAC_PAD = 2048          # 16 * 128
ND_OP = N_OP // NC        # 6250 dst rows per core
ND_MAC = N_MAC // NC      # 250
T_OP = (ND_OP + P - 1) // P    # 49 tiles (last has 106 rows)
T_MAC = (ND_MAC + P - 1) // P  # 2 tiles (last has 122 rows)

_last_results = [None]


def _proj(att):
    """att [H, 2*DK+1] -> (P_src [OUT,H], P_dst [OUT,H], attc [H])"""
    ps = np.zeros((OUT, HEADS), np.float32)
    pd = np.zeros((OUT, HEADS), np.float32)
    for h in range(HEADS):
        ps[h * DK:(h + 1) * DK, h] = att[h, :DK]
        pd[h * DK:(h + 1) * DK, h] = att[h, DK:2 * DK]
    return ps, pd, np.ascontiguousarray(att[:, 2 * DK]).astype(np.float32)


def _prep_edges(src, dst, feat, nd_core, n_tiles):
    """Sort by dst, shard dst ranges across cores, tile into 128-dst tiles,
    pad each tile's edge list to a block count common across cores.
    Returns per-tile common block counts + per-core packed arrays
    [128, nblk] layouts flattened: value(p,b) = edge b*128+p."""
    order = np.argsort(dst, kind='stable')
    src, dst, feat = src[order], dst[order], feat[order]
    core_lists = []
    for c in range(NC):
        lo, hi = c * nd_core, (c + 1) * nd_core
        a = np.searchsorted(dst, lo)
        b = np.searchsorted(dst, hi)
        s_c, d_c, f_c = src[a:b], dst[a:b] - lo, feat[a:b]
        tiles = []
        for t in range(n_tiles):
            ta = np.searchsorted(d_c, t * P)
            tb = np.searchsorted(d_c, (t + 1) * P)
            tiles.append((s_c[ta:tb], d_c[ta:tb] - t * P, f_c[ta:tb]))
        core_lists.append(tiles)
    nblk = [max(1, max((len(core_lists[c][t][0]) + P - 1) // P
                       for c in range(NC))) for t in range(n_tiles)]
    packed = []   # per core: dict of arrays
    for c in range(NC):
        lo = c * nd_core
        zs, ds, dr, ft = [], [], [], []
        for t in range(n_tiles):
            s_t, drel_t, f_t = core_lists[c][t]
            ne = nblk[t] * P
            pad = ne - len(s_t)
            s_p = np.concatenate([s_t, np.zeros(pad, s_t.dtype)])
            dd = core_lists[c][t][0]  # unused
            drel_p = np.concatenate([drel_t.astype(np.float32),
                                     np.full(pad, 999.0, np.float32)])
            dglob_p = np.concatenate([drel_t + t * P + lo,  # global dst id
                                      np.zeros(pad, drel_t.dtype)])
            f_p = np.concatenate([f_t.astype(np.float32),
                                  np.zeros(pad, np.float32)])
            # [P, nblk] with (p, b) = edge b*128+p
            zs.append(s_p.reshape(nblk[t], P).T)
            ds.append(dglob_p.reshape(nblk[t], P).T)
            dr.append(drel_p.reshape(nblk[t], P).T)
            ft.append(f_p.reshape(nblk[t], P).T)
        packed.append({
            'zidx': np.concatenate([x.reshape(P, -1) for x in zs], 1).astype(np.int32),
            'dloc': np.concatenate([x.reshape(P, -1) for x in ds], 1).astype(np.int32),
            'drel': np.concatenate([x.reshape(P, -1) for x in dr], 1).astype(np.float32),
            'feat': np.concatenate([x.reshape(P, -1) for x in ft], 1).astype(np.float32),
        })
    return nblk, packed


def build_kernel(nblk_seq, nblk_mo, nblk_om):
    nc = bacc.Bacc()
    NB_SEQ, NB_MO, NB_OM = sum(nblk_seq), sum(nblk_mo), sum(nblk_om)

    hT_op = nc.dram_tensor("hT_op", (IN_OP + 1, N_OP_PAD), F32, kind="ExternalInput")
    hT_mac = nc.dram_tensor("hT_mac", (IN_MAC + 1, N_MAC_PAD), F32, kind="ExternalInput")
    hT_own_op = nc.dram_tensor("hT_own_op", (IN_OP + 1, T_OP * P), F32, kind="ExternalInput")
    hT_own_mac = nc.dram_tensor("hT_own_mac", (IN_MAC + 1, T_MAC * P), F32, kind="ExternalInput")
    Wc_op = nc.dram_tensor("Wc_op", (IN_OP + 1, 144), F32, kind="ExternalInput")
    Wc_mac = nc.dram_tensor("Wc_mac", (IN_MAC + 1, 144), F32, kind="ExternalInput")
    consts = nc.dram_tensor("consts", (P, P + 12 + 4 * P), F32, kind="ExternalInput")
    edge_in = {}
    for nm, nb in (("seq", NB_SEQ), ("mo", NB_MO), ("om", NB_OM)):
        for role, dt in (("zidx", I32), ("dloc", I32), ("drel", F32), ("feat", F32)):
            edge_in[f"{nm}_{role}"] = nc.dram_tensor(
                f"{nm}_{role}", (P, nb), dt, kind="ExternalInput")
    out_op = nc.dram_tensor("out_op", (ND_OP, OUT), F32, kind="ExternalOutput")
    out_mac = nc.dram_tensor("out_mac", (ND_MAC, OUT), F32, kind="ExternalOutput")

    with tile.TileContext(nc) as tc:
        with (
            tc.tile_pool(name="con", bufs=1) as con,
            tc.tile_pool(name="sba", bufs=3) as sba,
            tc.tile_pool(name="sbe", bufs=2) as sbe,
            tc.tile_pool(name="sbg", bufs=4) as sbg,
            tc.tile_pool(name="sbom", bufs=1) as sbom,
            tc.tile_pool(name="ps", bufs=2, space="PSUM") as ps,
            tc.tile_pool(name="psd", bufs=2, space="PSUM") as psd,
            tc.tile_pool(name="dram", bufs=1, space="DRAM") as dr,
        ):
            # ---------------- constants ----------------
            ct = con.tile([P, P + 12 + 4 * P], F32)
            nc.sync.dma_start(out=ct[:], in_=consts[:, :])
            iota_t = ct[:, :P]
            attc = {"seq": ct[:, P:P + 4], "mo": ct[:, P + 4:P + 8],
                    "om": ct[:, P + 8:P + 12]}
            g_op = ct[:, P + 12:P + 12 + P]
            b_op = ct[:, P + 12 + P:P + 12 + 2 * P]
            g_mac = ct[:, P + 12 + 2 * P:P + 12 + 3 * P]
            b_mac = ct[:, P + 12 + 3 * P:P + 12 + 4 * P]

            wc_op = con.tile([IN_OP + 1, 144], F32)
            nc.sync.dma_start(out=wc_op[:], in_=Wc_op[:, :])
            wc_mac = con.tile([IN_MAC + 1, 144], F32)
            nc.sync.dma_start(out=wc_mac[:], in_=Wc_mac[:, :])

            # ---------------- phase A: tables ----------------
            z_op = dr.tile([N_OP_PAD, OUT], F32)
            s_op = dr.tile([N_OP_PAD, 16], F32)
            z_mac = dr.tile([N_MAC_PAD, OUT], F32)
            s_mac = dr.tile([N_MAC_PAD, 16], F32)

            def phase_a(hT, k_in, wc, n_pad, z_tab, s_tab):
                ntile = n_pad // P
                for j in range(0, ntile, 4):
                    cols = min(4, ntile - j)
                    ht = sba.tile([k_in, 4 * P], F32, tag="ht")
                    nc.sync.dma_start(out=ht[:, :cols * P],
                                      in_=hT[:, j * P:(j + cols) * P])
                    for q in range(cols):
                        i = j + q
                        pz = ps.tile([P, 144], F32, space="PSUM", tag="pz")
                        nc.tensor.matmul(pz[:], lhsT=ht[:, q * P:(q + 1) * P],
                                         rhs=wc[:], start=True, stop=True)
                        zs = sba.tile([P, 144], F32, tag="zs")
                        if i % 2 == 0:
                            nc.scalar.copy(zs[:], pz[:])
                        else:
                            nc.vector.tensor_copy(zs[:], pz[:])
                        nc.sync.dma_start(out=z_tab[i * P:(i + 1) * P, :],
                                          in_=zs[:, :OUT])
                        nc.sync.dma_start(out=s_tab[i * P:(i + 1) * P, :],
                                          in_=zs[:, OUT:144])

            phase_a(hT_op, IN_OP + 1, wc_op, N_OP_PAD, z_op, s_op)
            phase_a(hT_mac, IN_MAC + 1, wc_mac, N_MAC_PAD, z_mac, s_mac)

            # ---------------- phase B ----------------
            TYPES = {
                "seq": dict(ztab=z_op, s_src=s_op, so=0, s_dst=s_op, do=4),
                "mo":  dict(ztab=z_mac, s_src=s_mac, so=0, s_dst=s_op, do=12),
                "om":  dict(ztab=z_op, s_src=s_op, so=8, s_dst=s_mac, do=4),
            }
            off = {"seq": 0, "mo": 0, "om": 0}

            def do_type(nm, t, nb, p_acc):
                """pass1+pass2 for edge type nm on dst-tile t; accumulate
                weighted sums into p_acc. Returns r tile [P,4]."""
                cfg = TYPES[nm]
                o = off[nm]
                il = sbe.tile([P, 4 * nb], F32, tag="il")
                nc.sync.dma_start(out=il[:, 0 * nb:1 * nb].bitcast(I32),
                                  in_=edge_in[f"{nm}_zidx"][:, o:o + nb])
                nc.sync.dma_start(out=il[:, 1 * nb:2 * nb].bitcast(I32),
                                  in_=edge_in[f"{nm}_dloc"][:, o:o + nb])
                nc.sync.dma_start(out=il[:, 2 * nb:3 * nb],
                                  in_=edge_in[f"{nm}_drel"][:, o:o + nb])
                nc.sync.dma_start(out=il[:, 3 * nb:4 * nb],
                                  in_=edge_in[f"{nm}_feat"][:, o:o + nb])
                zidx = il[:, 0 * nb:1 * nb].bitcast(I32)
                dloc = il[:, 1 * nb:2 * nb].bitcast(I32)
                drel = il[:, 2 * nb:3 * nb]
                feat = il[:, 3 * nb:4 * nb]

                ss = sbe.tile([P, 16 * nb], F32, tag="ssrc")
                sd = sbe.tile([P, 16 * nb], F32, tag="sdst")
                for b in range(nb):
                    nc.gpsimd.indirect_dma_start(
                        out=ss[:, 16 * b:16 * (b + 1)], out_offset=None,
                        in_=cfg["s_src"][:, :],
                        in_offset=bass.IndirectOffsetOnAxis(ap=zidx[:, b:b + 1], axis=0))
                    nc.gpsimd.indirect_dma_start(
                        out=sd[:, 16 * b:16 * (b + 1)], out_offset=None,
                        in_=cfg["s_dst"][:, :],
                        in_offset=bass.IndirectOffsetOnAxis(ap=dloc[:, b:b + 1], axis=0))
                ss3 = ss[:].rearrange("p (n s) -> p n s", n=nb)[:, :, cfg["so"]:cfg["so"] + 4]
                sd3 = sd[:].rearrange("p (n s) -> p n s", n=nb)[:, :, cfg["do"]:cfg["do"] + 4]

                al = sbe.tile([P, 4 * nb], F32, tag="alpha")
                t0 = sbe.tile([P, 4 * nb], F32, tag="sc0")
                # fc = feat * attc
                nc.vector.tensor_tensor(
                    out=t0[:], in0=feat.unsqueeze(2).to_broadcast([P, nb, 4]),
                    in1=attc[nm].unsqueeze(1).to_broadcast([P, nb, 4]), op=OP.mult)
                nc.vector.tensor_tensor(out=al[:], in0=ss3, in1=sd3, op=OP.add)
                nc.vector.tensor_tensor(out=al[:], in0=al[:], in1=t0[:], op=OP.add)
                # leaky relu 0.2 + clip [-20, 20]
                nc.vector.tensor_scalar(out=t0[:], in0=al[:], scalar1=0.2,
                                        scalar2=None, op0=OP.mult)
                nc.vector.tensor_tensor(out=al[:], in0=al[:], in1=t0[:], op=OP.max)
                nc.vector.tensor_scalar(out=al[:], in0=al[:], scalar1=20.0,
                                        scalar2=-20.0, op0=OP.min, op1=OP.max)
                nc.scalar.activation(al[:], al[:], AF.Exp)
                al16 = sbe.tile([P, 4 * nb], BF16, tag="al16")
                nc.vector.tensor_copy(al16[:], al[:])

                # S matrices (one batched is_equal), denominator matmuls
                pool_s = sbom if nm == "om" else sbe
                S = pool_s.tile([P, nb * P], BF16, tag=("Som" if nm == "om" else "S"))
                nc.vector.tensor_tensor(
                    out=S[:],
                    in0=iota_t.unsqueeze(1).to_broadcast([P, nb, P]),
                    in1=drel.unsqueeze(2).to_broadcast([P, nb, P]),
                    op=OP.is_equal)
                pd_ = psd.tile([P, 4], F32, space="PSUM", tag="pd")
                for b in range(nb):
                    nc.tensor.matmul(pd_[:], lhsT=S[:, b * P:(b + 1) * P],
                                     rhs=al16[:, 4 * b:4 * (b + 1)],
                                     start=(b == 0), stop=(b == nb - 1))
                r = sbe.tile([P, 4], F32, tag="r")
                nc.vector.tensor_scalar(out=r[:], in0=pd_[:], scalar1=EPS,
                                        scalar2=None, op0=OP.add)
                nc.vector.reciprocal(r[:], r[:])

                # pass 2: gather z rows, scale by alpha, matmul-accumulate
                for b in range(nb):
                    zg = sbg.tile([P, OUT], F32, tag="zg")
                    nc.gpsimd.indirect_dma_start(
                        out=zg[:], out_offset=None, in_=cfg["ztab"][:, :],
                        in_offset=bass.IndirectOffsetOnAxis(ap=zidx[:, b:b + 1], axis=0))
                    zsc = sbg.tile([P, OUT], BF16, tag="zsc")
                    nc.vector.tensor_tensor(
                        out=zsc[:], in0=zg[:],
                        in1=al16[:, 4 * b:4 * (b + 1)].unsqueeze(2).to_broadcast([P, 4, DK]),
                        op=OP.mult)
                    nc.tensor.matmul(p_acc[:], lhsT=S[:, b * P:(b + 1) * P],
                                     rhs=zsc[:], start=(b == 0), stop=(b == nb - 1))
                off[nm] += nb
                return r

            def post(p_parts, rs, p_z, g_rep, b_rep, out_t, rows, row0):
                """combine attention parts (scaled by r), residual, LN, ELU."""
                res = sbe.tile([P, OUT], F32, tag="res")
                acc = None
                for (p_acc, r) in zip(p_parts, rs):
                    tscale = sbe.tile([P, OUT], F32, tag="tsc")
                    nc.vector.tensor_tensor(
                        out=tscale[:], in0=p_acc[:],
                        in1=r[:].unsqueeze(2).to_broadcast([P, 4, DK]), op=OP.mult)
                    if acc is None:
                        acc = tscale
                    else:
                        nc.vector.tensor_tensor(out=tscale[:], in0=acc[:],
                                                in1=tscale[:], op=OP.add)
                        acc = tscale
                nc.vector.tensor_tensor(out=res[:], in0=acc[:], in1=p_z[:], op=OP.add)
                # layernorm
                mu = sbe.tile([P, 1], F32, tag="mu")
                nc.vector.tensor_reduce(out=mu[:], in_=res[:],
                                        axis=mybir.AxisListType.X, op=OP.add)
                nc.vector.tensor_scalar(out=mu[:], in0=mu[:], scalar1=-1.0 / OUT,
                                        scalar2=None, op0=OP.mult)
                xm = sbe.tile([P, OUT], F32, tag="xm")
                nc.scalar.activation(xm[:], res[:], AF.Identity, bias=mu[:, :1])
                sq = sbe.tile([P, OUT], F32, tag="sq")
                nc.scalar.activation(sq[:], xm[:], AF.Square)
                vs = sbe.tile([P, 1], F32, tag="vs")
                nc.vector.tensor_reduce(out=vs[:], in_=sq[:],
                                        axis=mybir.AxisListType.X, op=OP.add)
                nc.vector.tensor_scalar(out=vs[:], in0=vs[:], scalar1=1.0 / OUT,
                                        scalar2=LN_EPS, op0=OP.mult, op1=OP.add)
                nc.scalar.activation(vs[:], vs[:], AF.Sqrt)
                nc.vector.reciprocal(vs[:], vs[:])
                nc.vector.tensor_scalar(out=xm[:], in0=xm[:], scalar1=vs[:, :1],
                                        scalar2=None, op0=OP.mult)
                nc.vector.tensor_tensor(out=xm[:], in0=xm[:], in1=g_rep, op=OP.mult)
                nc.vector.tensor_tensor(out=xm[:], in0=xm[:], in1=b_rep, op=OP.add)
                # elu(x) = max(x, exp(min(x,0)) - 1)
                el = sbe.tile([P, OUT], F32, tag="el")
                nc.vector.tensor_scalar(out=el[:], in0=xm[:], scalar1=0.0,
                                        scalar2=None, op0=OP.min)
                nc.scalar.activation(el[:], el[:], AF.Exp)
                nc.vector.tensor_scalar(out=el[:], in0=el[:], scalar1=-1.0,
                                        scalar2=None, op0=OP.add)
                nc.vector.tensor_tensor(out=el[:], in0=xm[:], in1=el[:], op=OP.max)
                nc.sync.dma_start(out=out_t[row0:row0 + rows, :], in_=el[:rows, :])

            # op dst tiles
            for t in range(T_OP):
                p_z = ps.tile([P, OUT], F32, space="PSUM", tag="pz")
                hto = sba.tile([IN_OP + 1, P], F32, tag="hto")
                nc.sync.dma_start(out=hto[:], in_=hT_own_op[:, t * P:(t + 1) * P])
                nc.tensor.matmul(p_z[:], lhsT=hto[:], rhs=wc_op[:, :OUT],
                                 start=True, stop=True)
                p_seq = ps.tile([P, OUT], F32, space="PSUM", tag="pseq")
                r_seq = do_type("seq", t, nblk_seq[t], p_seq)
                p_mo = ps.tile([P, OUT], F32, space="PSUM", tag="pmo")
                r_mo = do_type("mo", t, nblk_mo[t], p_mo)
                rows = min(P, ND_OP - t * P)
                post([p_seq, p_mo], [r_seq, r_mo], p_z, g_op, b_op,
                     out_op, rows, t * P)

            # mac dst tiles
            for t in range(T_MAC):
                p_z = ps.tile([P, OUT], F32, space="PSUM", tag="pz")
                htm = sba.tile([IN_MAC + 1, P], F32, tag="htm")
                nc.sync.dma_start(out=htm[:], in_=hT_own_mac[:, t * P:(t + 1) * P])
                nc.tensor.matmul(p_z[:], lhsT=htm[:], rhs=wc_mac[:, :OUT],
                                 start=True, stop=True)
                p_om = ps.tile([P, OUT], F32, space="PSUM", tag="pseq")
                r_om = do_type("om", t, nblk_om[t], p_om)
                rows = min(P, ND_MAC - t * P)
                post([p_om], [r_om], p_z, g_mac, b_mac, out_mac, rows, t * P)

    nc.finalize()
    return nc


def kernel(h_op, h_mac, seq_src, seq_dst, op_mac_src, op_mac_dst,
           mac_op_src, mac_op_dst, feat_seq, feat_op_mac, feat_mac_op,
           W_op_w, W_op_b, W_mac_w, W_mac_b,
           att_seq, att_op_mac, att_mac_op,
           ln_op_g, ln_op_b, ln_mac_g, ln_mac_b):
    tonp = lambda x: np.asarray(x)
    h_op, h_mac = tonp(h_op).astype(np.float32), tonp(h_mac).astype(np.float32)
    idxs = [tonp(x).astype(np.int64) for x in
            (seq_src, seq_dst, op_mac_src, op_mac_dst, mac_op_src, mac_op_dst)]
    seq_src, seq_dst, op_mac_src, op_mac_dst, mac_op_src, mac_op_dst = idxs
    feat_seq = tonp(feat_seq).astype(np.float32).ravel()
    feat_op_mac = tonp(feat_op_mac).astype(np.float32).ravel()
    feat_mac_op = tonp(feat_mac_op).astype(np.float32).ravel()
    W_op_w, W_op_b = tonp(W_op_w).astype(np.float32), tonp(W_op_b).astype(np.float32)
    W_mac_w, W_mac_b = tonp(W_mac_w).astype(np.float32), tonp(W_mac_b).astype(np.float32)
    att_seq, att_op_mac, att_mac_op = (tonp(att_seq).astype(np.float32),
                                       tonp(att_op_mac).astype(np.float32),
                                       tonp(att_mac_op).astype(np.float32))

    # ---- host param prep ----
    ps_seq, pd_seq, ac_seq = _proj(att_seq)
    ps_om, pd_om, ac_om = _proj(att_op_mac)
    ps_mo, pd_mo, ac_mo = _proj(att_mac_op)
    WT_op = W_op_w.T                      # [64,128]
    WT_mac = W_mac_w.T                    # [32,128]
    # s-table cols for op nodes: src_seq, dst_seq, src_om, dst_mo
    Q_op = np.concatenate([WT_op @ ps_seq, WT_op @ pd_seq,
                           WT_op @ ps_om, WT_op @ pd_mo], 1)      # [64,16]
    bq_op = np.concatenate([W_op_b @ ps_seq, W_op_b @ pd_seq,
                            W_op_b @ ps_om, W_op_b @ pd_mo])      # [16]
    # s-table cols for mac nodes: src_mo, dst_om (rest zero)
    Q_mac = np.concatenate([WT_mac @ ps_mo, WT_mac @ pd_om,
                            np.zeros((IN_MAC, 8), np.float32)], 1)
    bq_mac = np.concatenate([W_mac_b @ ps_mo, W_mac_b @ pd_om,
                             np.zeros(8, np.float32)])
    Wc_op = np.zeros((IN_OP + 1, 144), np.float32)
    Wc_op[:IN_OP, :OUT] = WT_op
    Wc_op[IN_OP, :OUT] = W_op_b
    Wc_op[:IN_OP, OUT:] = Q_op
    Wc_op[IN_OP, OUT:] = bq_op
    Wc_mac = np.zeros((IN_MAC + 1, 144), np.float32)
    Wc_mac[:IN_MAC, :OUT] = WT_mac
    Wc_mac[IN_MAC, :OUT] = W_mac_b
    Wc_mac[:IN_MAC, OUT:] = Q_mac
    Wc_mac[IN_MAC, OUT:] = bq_mac

    hT_op = np.zeros((IN_OP + 1, N_OP_PAD), np.float32)
    hT_op[:IN_OP, :N_OP] = h_op.T
    hT_op[IN_OP, :] = 1.0
    hT_mac = np.zeros((IN_MAC + 1, N_MAC_PAD), np.float32)
    hT_mac[:IN_MAC, :N_MAC] = h_mac.T
    hT_mac[IN_MAC, :] = 1.0

    consts_base = np.zeros((P, P + 12 + 4 * P), np.float32)
    consts_base[:, :P] = np.arange(P, dtype=np.float32)[None, :]
    consts_base[:, P:P + 4] = ac_seq[None, :]
    consts_base[:, P + 4:P + 8] = ac_mo[None, :]
    consts_base[:, P + 8:P + 12] = ac_om[None, :]
    consts_base[:, P + 12:P + 12 + P] = tonp(ln_op_g).astype(np.float32)[None, :]
    consts_base[:, P + 12 + P:P + 12 + 2 * P] = tonp(ln_op_b).astype(np.float32)[None, :]
    consts_base[:, P + 12 + 2 * P:P + 12 + 3 * P] = tonp(ln_mac_g).astype(np.float32)[None, :]
    consts_base[:, P + 12 + 3 * P:P + 12 + 4 * P] = tonp(ln_mac_b).astype(np.float32)[None, :]

    nblk_seq, pk_seq = _prep_edges(seq_src, seq_dst, feat_seq, ND_OP, T_OP)
    nblk_mo, pk_mo = _prep_edges(mac_op_src, mac_op_dst, feat_mac_op, ND_OP, T_OP)
    nblk_om, pk_om = _prep_edges(op_mac_src, op_mac_dst, feat_op_mac, ND_MAC, T_MAC)

    in_maps = []
    for c in range(NC):
        m = {"hT_op": hT_op, "hT_mac": hT_mac,
             "Wc_op": Wc_op, "Wc_mac": Wc_mac, "consts": consts_base}
        ho = np.zeros((IN_OP + 1, T_OP * P), np.float32)
        ho[:, :ND_OP] = hT_op[:, c * ND_OP:(c + 1) * ND_OP]
        m["hT_own_op"] = ho
        hm = np.zeros((IN_MAC + 1, T_MAC * P), np.float32)
        hm[:, :ND_MAC] = hT_mac[:, c * ND_MAC:(c + 1) * ND_MAC]
        m["hT_own_mac"] = hm
        for nm, pk in (("seq", pk_seq), ("mo", pk_mo), ("om", pk_om)):
            m[f"{nm}_zidx"] = pk[c]['zidx']
            m[f"{nm}_dloc"] = pk[c]['dloc']
            m[f"{nm}_drel"] = pk[c]['drel']
            m[f"{nm}_feat"] = pk[c]['feat']
        in_maps.append(m)

    nc = build_kernel(nblk_seq, nblk_mo, nblk_om)
    import os
    res = bass_utils.run_bass_kernel_spmd(
        nc, in_maps, core_ids=list(range(NC)),
        trace=bool(os.environ.get("BASS_TRACE")))
    _last_results[0] = res
    res_op = np.concatenate([res.results[c]["out_op"] for c in range(NC)], 0)
    res_mac = np.concatenate([res.results[c]["out_mac"] for c in range(NC)], 0)
    return res_op, res_mac


# revision 7
# speedup vs baseline: 1.3863x; 1.3513x over previous
"""HeteroGAT layer Trainium kernel: 8-core dst-sharded edge-parallel.

Strategy: sort each edge set by destination on the host, shard destinations
across the 8 cores (each core owns a contiguous dst range -> no collectives).
On device: phase A computes projected embeddings z = h @ W.T + b and per-node
attention score tables into internal DRAM; phase B processes 128-dst tiles,
building per-128-edge-block one-hot selection matrices (is_equal vs iota) and
doing the two segment sums (denominator, weighted) as PE matmuls into PSUM.
Residual z is recomputed into PSUM per tile; LN + ELU fused on the way out.
"""

import sys
import types
import numpy as np

# -- inject missing antenv.axon_hooks so NTFF tracing works under axon -------
if 'antenv.axon_hooks' not in sys.modules:
    _m = types.ModuleType('antenv.axon_hooks')
    _hh = [None]
    _m.set_axon_ntff_profile_hook = lambda h: _hh.__setitem__(0, h)
    _m.get_axon_ntff_profile_hook = lambda: _hh[0]
    sys.modules['antenv.axon_hooks'] = _m
    try:
        import antenv
        antenv.axon_hooks = _m
        from trn_agent_boot.trn_boot import _ntff_profile_via_ctypes
        _m.set_axon_ntff_profile_hook(
            _ntff_profile_via_ctypes('/opt/axon/libaxon_pjrt.so'))
    except Exception:
        pass

import concourse.bass as bass
import concourse.bacc as bacc
import concourse.mybir as mybir
import concourse.tile as tile
from concourse import bass_utils

F32 = mybir.dt.float32
BF16 = mybir.dt.bfloat16
I32 = mybir.dt.int32
AF = mybir.ActivationFunctionType
OP = mybir.AluOpType

P = 128
NC = 8
N_OP, N_MAC = 50000, 2000
IN_OP, IN_MAC, OUT, HEADS, DK = 64, 32, 128, 4, 32
EPS, LN_EPS = 1e-6, 1e-5
N_OP_PAD = 50176          # 392 * 128
N_MAC_PAD = 2048          # 16 * 128
ND_OP = N_OP // NC        # 6250 dst rows per core
ND_MAC = N_MAC // NC      # 250
T_OP = (ND_OP + P - 1) // P    # 49 tiles (last has 106 rows)
T_MAC = (ND_MAC + P - 1) // P  # 2 tiles (last has 122 rows)

_last_results = [None]


def _proj(att):
    """att [H, 2*DK+1] -> (P_src [OUT,H], P_dst [OUT,H], attc [H])"""
    ps = np.zeros((OUT, HEADS), np.float32)
    pd = np.zeros((OUT, HEADS), np.float32)
    for h in range(HEADS):
        ps[h * DK:(h + 1) * DK, h] = att[h, :DK]
        pd[h * DK:(h + 1) * DK, h] = att[h, DK:2 * DK]
    return ps, pd, np.ascontiguousarray(att[:, 2 * DK]).astype(np.float32)


def _prep_edges(src, dst, feat, nd_core, n_tiles):
    """Sort by dst, shard dst ranges across cores, tile into 128-dst tiles,
    pad each tile's edge list to a block count common across cores.
    Returns per-tile common block counts + per-core packed arrays
    [128, nblk] layouts flattened: value(p,b) = edge b*128+p."""
    order = np.argsort(dst, kind='stable')
    src, dst, feat = src[order], dst[order], feat[order]
    core_lists = []
    for c in range(NC):
        lo, hi = c * nd_core, (c + 1) * nd_core
        a = np.searchsorted(dst, lo)
        b = np.searchsorted(dst, hi)
        s_c, d_c, f_c = src[a:b], dst[a:b] - lo, feat[a:b]
        tiles = []
        for t in range(n_tiles):
            ta = np.searchsorted(d_c, t * P)
            tb = np.searchsorted(d_c, (t + 1) * P)
            tiles.append((s_c[ta:tb], d_c[ta:tb] - t * P, f_c[ta:tb]))
        core_lists.append(tiles)
    nblk = [max(1, max((len(core_lists[c][t][0]) + P - 1) // P
                       for c in range(NC))) for t in range(n_tiles)]
    packed = []   # per core: dict of arrays
    for c in range(NC):
        lo = c * nd_core
        zs, ds, dr, ft = [], [], [], []
        for t in range(n_tiles):
            s_t, drel_t, f_t = core_lists[c][t]
            ne = nblk[t] * P
            pad = ne - len(s_t)
            s_p = np.concatenate([s_t, np.zeros(pad, s_t.dtype)])
            dd = core_lists[c][t][0]  # unused
            drel_p = np.concatenate([drel_t.astype(np.float32),
                                     np.full(pad, 999.0, np.float32)])
            dglob_p = np.concatenate([drel_t + t * P + lo,  # global dst id
                                      np.zeros(pad, drel_t.dtype)])
            f_p = np.concatenate([f_t.astype(np.float32),
                                  np.zeros(pad, np.float32)])
            # [P, nblk] with (p, b) = edge b*128+p
            zs.append(s_p.reshape(nblk[t], P).T)
            ds.append(dglob_p.reshape(nblk[t], P).T)
            dr.append(drel_p.reshape(nblk[t], P).T)
            ft.append(f_p.reshape(nblk[t], P).T)
        packed.append({
            'zidx': np.concatenate([x.reshape(P, -1) for x in zs], 1).astype(np.int32),
            'dloc': np.concatenate([x.reshape(P, -1) for x in ds], 1).astype(np.int32),
            'drel': np.concatenate([x.reshape(P, -1) for x in dr], 1).astype(np.float32),
            'feat': np.concatenate([x.reshape(P, -1) for x in ft], 1).astype(np.float32),
        })
    return nblk, packed


def build_kernel(nblk_seq, nblk_mo, nblk_om):
    nc = bacc.Bacc()
    NB_SEQ, NB_MO, NB_OM = sum(nblk_seq), sum(nblk_mo), sum(nblk_om)

    hT_op = nc.dram_tensor("hT_op", (IN_OP + 1, N_OP_PAD), F32, kind="ExternalInput")
    hT_mac = nc.dram_tensor("hT_mac", (IN_MAC + 1, N_MAC_PAD), F32, kind="ExternalInput")
    hT_own_op = nc.dram_tensor("hT_own_op", (IN_OP + 1, T_OP * P), F32, kind="ExternalInput")
    hT_own_mac = nc.dram_tensor("hT_own_mac", (IN_MAC + 1, T_MAC * P), F32, kind="ExternalInput")
    Wc_op = nc.dram_tensor("Wc_op", (IN_OP + 1, 144), F32, kind="ExternalInput")
    Wc_mac = nc.dram_tensor("Wc_mac", (IN_MAC + 1, 144), F32, kind="ExternalInput")
    consts = nc.dram_tensor("consts", (P, P + 12 + 4 * P), F32, kind="ExternalInput")
    edge_in = {}
    for nm, nb in (("seq", NB_SEQ), ("mo", NB_MO), ("om", NB_OM)):
        for role, dt in (("zidx", I32), ("dloc", I32), ("drel", F32), ("feat", F32)):
            edge_in[f"{nm}_{role}"] = nc.dram_tensor(
                f"{nm}_{role}", (P, nb), dt, kind="ExternalInput")
    out_op = nc.dram_tensor("out_op", (ND_OP, OUT), F32, kind="ExternalOutput")
    out_mac = nc.dram_tensor("out_mac", (ND_MAC, OUT), F32, kind="ExternalOutput")

    with tile.TileContext(nc) as tc:
        with (
            tc.tile_pool(name="con", bufs=1) as con,
            tc.tile_pool(name="sba", bufs=3) as sba,
            tc.tile_pool(name="sbe", bufs=2) as sbe,
            tc.tile_pool(name="sbg", bufs=4) as sbg,
            tc.tile_pool(name="sbom", bufs=1) as sbom,
            tc.tile_pool(name="ps", bufs=2, space="PSUM") as ps,
            tc.tile_pool(name="psd", bufs=2, space="PSUM") as psd,
            tc.tile_pool(name="dram", bufs=1, space="DRAM") as dr,
        ):
            # ---------------- constants ----------------
            ct = con.tile([P, P + 12 + 4 * P], F32)
            nc.sync.dma_start(out=ct[:], in_=consts[:, :])
            iota_t = ct[:, :P]
            attc = {"seq": ct[:, P:P + 4], "mo": ct[:, P + 4:P + 8],
                    "om": ct[:, P + 8:P + 12]}
            g_op = ct[:, P + 12:P + 12 + P]
            b_op = ct[:, P + 12 + P:P + 12 + 2 * P]
            g_mac = ct[:, P + 12 + 2 * P:P + 12 + 3 * P]
            b_mac = ct[:, P + 12 + 3 * P:P + 12 + 4 * P]

            wc_op = con.tile([IN_OP + 1, 144], F32)
            nc.sync.dma_start(out=wc_op[:], in_=Wc_op[:, :])
            wc_mac = con.tile([IN_MAC + 1, 144], F32)
            nc.sync.dma_start(out=wc_mac[:], in_=Wc_mac[:, :])

            # ---------------- phase A: tables ----------------
            z_op = dr.tile([N_OP_PAD, 144], F32)
            s_op = dr.tile([N_OP_PAD, 16], F32)
            z_mac = dr.tile([N_MAC_PAD, 144], F32)
            s_mac = dr.tile([N_MAC_PAD, 16], F32)

            def phase_a(hT, k_in, wc, n_pad, z_tab, s_tab):
                ntile = n_pad // P
                for j in range(0, ntile, 4):
                    cols = min(4, ntile - j)
                    ht = sba.tile([k_in, 4 * P], F32, tag="ht")
                    nc.sync.dma_start(out=ht[:, :cols * P],
                                      in_=hT[:, j * P:(j + cols) * P])
                    for q in range(cols):
                        i = j + q
                        pz = ps.tile([P, 144], F32, space="PSUM", tag="pz")
                        nc.tensor.matmul(pz[:], lhsT=ht[:, q * P:(q + 1) * P],
                                         rhs=wc[:], start=True, stop=True)
                        zs = sba.tile([P, 144], F32, tag="zs")
                        if i % 2 == 0:
                            nc.scalar.copy(zs[:], pz[:])
                        else:
                            nc.vector.tensor_copy(zs[:], pz[:])
                        nc.sync.dma_start(out=z_tab[i * P:(i + 1) * P, :],
                                          in_=zs[:, :])
                        nc.sync.dma_start(out=s_tab[i * P:(i + 1) * P, :],
                                          in_=zs[:, OUT:144])

            phase_a(hT_op, IN_OP + 1, wc_op, N_OP_PAD, z_op, s_op)
            phase_a(hT_mac, IN_MAC + 1, wc_mac, N_MAC_PAD, z_mac, s_mac)

            # ---------------- phase B ----------------
            TYPES = {
                "seq": dict(ztab=z_op, s_src=s_op, so=0, s_dst=s_op, do=4),
                "mo":  dict(ztab=z_mac, s_src=s_mac, so=0, s_dst=s_op, do=12),
                "om":  dict(ztab=z_op, s_src=s_op, so=8, s_dst=s_mac, do=4),
            }
            off = {"seq": 0, "mo": 0, "om": 0}

            def do_type(nm, t, nb, p_acc):
                """pass1+pass2 for edge type nm on dst-tile t; accumulate
                weighted sums into p_acc. Returns r tile [P,4]."""
                cfg = TYPES[nm]
                o = off[nm]
                il = sbe.tile([P, 4 * nb], F32, tag="il")
                nc.sync.dma_start(out=il[:, 0 * nb:1 * nb].bitcast(I32),
                                  in_=edge_in[f"{nm}_zidx"][:, o:o + nb])
                nc.sync.dma_start(out=il[:, 1 * nb:2 * nb].bitcast(I32),
                                  in_=edge_in[f"{nm}_dloc"][:, o:o + nb])
                nc.sync.dma_start(out=il[:, 2 * nb:3 * nb],
                                  in_=edge_in[f"{nm}_drel"][:, o:o + nb])
                nc.sync.dma_start(out=il[:, 3 * nb:4 * nb],
                                  in_=edge_in[f"{nm}_feat"][:, o:o + nb])
                zidx = il[:, 0 * nb:1 * nb].bitcast(I32)
                dloc = il[:, 1 * nb:2 * nb].bitcast(I32)
                drel = il[:, 2 * nb:3 * nb]
                feat = il[:, 3 * nb:4 * nb]

                zcpool = sbom if nm == "om" else sbe
                zc = zcpool.tile([P, 144 * nb], F32,
                                 tag=("zcom" if nm == "om" else "zc"))
                sd = sbe.tile([P, 16 * nb], F32, tag="sdst")
                for b in range(nb):
                    nc.gpsimd.indirect_dma_start(
                        out=zc[:, 144 * b:144 * (b + 1)], out_offset=None,
                        in_=cfg["ztab"][:, :],
                        in_offset=bass.IndirectOffsetOnAxis(ap=zidx[:, b:b + 1], axis=0))
                    nc.gpsimd.indirect_dma_start(
                        out=sd[:, 16 * b:16 * (b + 1)], out_offset=None,
                        in_=cfg["s_dst"][:, :],
                        in_offset=bass.IndirectOffsetOnAxis(ap=dloc[:, b:b + 1], axis=0))
                zc3 = zc[:].rearrange("p (n s) -> p n s", n=nb)
                ss3 = zc3[:, :, OUT + cfg["so"]:OUT + cfg["so"] + 4]
                sd3 = sd[:].rearrange("p (n s) -> p n s", n=nb)[:, :, cfg["do"]:cfg["do"] + 4]

                al = sbe.tile([P, 4 * nb], F32, tag="alpha")
                t0 = sbe.tile([P, 4 * nb], F32, tag="sc0")
                # fc = feat * attc
                nc.vector.tensor_tensor(
                    out=t0[:], in0=feat.unsqueeze(2).to_broadcast([P, nb, 4]),
                    in1=attc[nm].unsqueeze(1).to_broadcast([P, nb, 4]), op=OP.mult)
                nc.vector.tensor_tensor(out=al[:], in0=ss3, in1=sd3, op=OP.add)
                nc.vector.tensor_tensor(out=al[:], in0=al[:], in1=t0[:], op=OP.add)
                # leaky relu 0.2 + clip [-20, 20]
                nc.vector.tensor_scalar(out=t0[:], in0=al[:], scalar1=0.2,
                                        scalar2=None, op0=OP.mult)
                nc.vector.tensor_tensor(out=al[:], in0=al[:], in1=t0[:], op=OP.max)
                nc.vector.tensor_scalar(out=al[:], in0=al[:], scalar1=20.0,
                                        scalar2=-20.0, op0=OP.min, op1=OP.max)
                nc.scalar.activation(al[:], al[:], AF.Exp)
                al16 = sbe.tile([P, 4 * nb], BF16, tag="al16")
                nc.vector.tensor_copy(al16[:], al[:])

                # S matrices (one batched is_equal), denominator matmuls
                pool_s = sbom if nm == "om" else sbe
                S = pool_s.tile([P, nb * P], BF16, tag=("Som" if nm == "om" else "S"))
                nc.vector.tensor_tensor(
                    out=S[:],
                    in0=iota_t.unsqueeze(1).to_broadcast([P, nb, P]),
                    in1=drel.unsqueeze(2).to_broadcast([P, nb, P]),
                    op=OP.is_equal)
                pd_ = psd.tile([P, 4], F32, space="PSUM", tag="pd")
                for b in range(nb):
                    nc.tensor.matmul(pd_[:], lhsT=S[:, b * P:(b + 1) * P],
                                     rhs=al16[:, 4 * b:4 * (b + 1)],
                                     start=(b == 0), stop=(b == nb - 1))
                r = sbe.tile([P, 4], F32, tag="r")
                nc.vector.tensor_scalar(out=r[:], in0=pd_[:], scalar1=EPS,
                                        scalar2=None, op0=OP.add)
                nc.vector.reciprocal(r[:], r[:])

                # pass 2: gather z rows, scale by alpha, matmul-accumulate
                for b in range(nb):
                    zsc = sbg.tile([P, OUT], BF16, tag="zsc")
                    nc.vector.tensor_tensor(
                        out=zsc[:], in0=zc[:, 144 * b:144 * b + OUT],
                        in1=al16[:, 4 * b:4 * (b + 1)].unsqueeze(2).to_broadcast([P, 4, DK]),
                        op=OP.mult)
                    nc.tensor.matmul(p_acc[:], lhsT=S[:, b * P:(b + 1) * P],
                                     rhs=zsc[:], start=(b == 0), stop=(b == nb - 1))
                off[nm] += nb
                return r

            def post(p_parts, rs, p_z, g_rep, b_rep, out_t, rows, row0):
                """combine attention parts (scaled by r), residual, LN, ELU."""
                res = sbe.tile([P, OUT], F32, tag="res")
                acc = None
                for (p_acc, r) in zip(p_parts, rs):
                    tscale = sbe.tile([P, OUT], F32, tag="tsc")
                    nc.vector.tensor_tensor(
                        out=tscale[:], in0=p_acc[:],
                        in1=r[:].unsqueeze(2).to_broadcast([P, 4, DK]), op=OP.mult)
                    if acc is None:
                        acc = tscale
                    else:
                        nc.vector.tensor_tensor(out=tscale[:], in0=acc[:],
                                                in1=tscale[:], op=OP.add)
                        acc = tscale
                nc.vector.tensor_tensor(out=res[:], in0=acc[:], in1=p_z[:], op=OP.add)
                # layernorm
                mu = sbe.tile([P, 1], F32, tag="mu")
                nc.vector.tensor_reduce(out=mu[:], in_=res[:],
                                        axis=mybir.AxisListType.X, op=OP.add)
                nc.vector.tensor_scalar(out=mu[:], in0=mu[:], scalar1=-1.0 / OUT,
                                        scalar2=None, op0=OP.mult)
                xm = sbe.tile([P, OUT], F32, tag="xm")
                nc.scalar.activation(xm[:], res[:], AF.Identity, bias=mu[:, :1])
                sq = sbe.tile([P, OUT], F32, tag="sq")
                nc.scalar.activation(sq[:], xm[:], AF.Square)
                vs = sbe.tile([P, 1], F32, tag="vs")
                nc.vector.tensor_reduce(out=vs[:], in_=sq[:],
                                        axis=mybir.AxisListType.X, op=OP.add)
                nc.vector.tensor_scalar(out=vs[:], in0=vs[:], scalar1=1.0 / OUT,
                                        scalar2=LN_EPS, op0=OP.mult, op1=OP.add)
                nc.scalar.activation(vs[:], vs[:], AF.Sqrt)
                nc.vector.reciprocal(vs[:], vs[:])
                nc.vector.tensor_scalar(out=xm[:], in0=xm[:], scalar1=vs[:, :1],
                                        scalar2=None, op0=OP.mult)
                nc.vector.tensor_tensor(out=xm[:], in0=xm[:], in1=g_rep, op=OP.mult)
                nc.vector.tensor_tensor(out=xm[:], in0=xm[:], in1=b_rep, op=OP.add)
                # elu(x) = max(x, exp(min(x,0)) - 1)
                el = sbe.tile([P, OUT], F32, tag="el")
                nc.vector.tensor_scalar(out=el[:], in0=xm[:], scalar1=0.0,
                                        scalar2=None, op0=OP.min)
                nc.scalar.activation(el[:], el[:], AF.Exp)
                nc.vector.tensor_scalar(out=el[:], in0=el[:], scalar1=-1.0,
                                        scalar2=None, op0=OP.add)
                nc.vector.tensor_tensor(out=el[:], in0=xm[:], in1=el[:], op=OP.max)
                nc.sync.dma_start(out=out_t[row0:row0 + rows, :], in_=el[:rows, :])

            # op dst tiles
            for t in range(T_OP):
                p_z = ps.tile([P, OUT], F32, space="PSUM", tag="pz")
                hto = sba.tile([IN_OP + 1, P], F32, tag="hto")
                nc.sync.dma_start(out=hto[:], in_=hT_own_op[:, t * P:(t + 1) * P])
                nc.tensor.matmul(p_z[:], lhsT=hto[:], rhs=wc_op[:, :OUT],
                                 start=True, stop=True)
                p_seq = ps.tile([P, OUT], F32, space="PSUM", tag="pseq")
                r_seq = do_type("seq", t, nblk_seq[t], p_seq)
                p_mo = ps.tile([P, OUT], F32, space="PSUM", tag="pmo")
                r_mo = do_type("mo", t, nblk_mo[t], p_mo)
                rows = min(P, ND_OP - t * P)
                post([p_seq, p_mo], [r_seq, r_mo], p_z, g_op, b_op,
                     out_op, rows, t * P)

            # mac dst tiles
            for t in range(T_MAC):
                p_z = ps.tile([P, OUT], F32, space="PSUM", tag="pz")
                htm = sba.tile([IN_MAC + 1, P], F32, tag="htm")
                nc.sync.dma_start(out=htm[:], in_=hT_own_mac[:, t * P:(t + 1) * P])
                nc.tensor.matmul(p_z[:], lhsT=htm[:], rhs=wc_mac[:, :OUT],
                                 start=True, stop=True)
                p_om = ps.tile([P, OUT], F32, space="PSUM", tag="pseq")
                r_om = do_type("om", t, nblk_om[t], p_om)
                rows = min(P, ND_MAC - t * P)
                post([p_om], [r_om], p_z, g_mac, b_mac, out_mac, rows, t * P)

    nc.finalize()
    return nc


def kernel(h_op, h_mac, seq_src, seq_dst, op_mac_src, op_mac_dst,
           mac_op_src, mac_op_dst, feat_seq, feat_op_mac, feat_mac_op,
           W_op_w, W_op_b, W_mac_w, W_mac_b,
           att_seq, att_op_mac, att_mac_op,
           ln_op_g, ln_op_b, ln_mac_g, ln_mac_b):
    tonp = lambda x: np.asarray(x)
    h_op, h_mac = tonp(h_op).astype(np.float32), tonp(h_mac).astype(np.float32)
    idxs = [tonp(x).astype(np.int64) for x in
            (seq_src, seq_dst, op_mac_src, op_mac_dst, mac_op_src, mac_op_dst)]
    seq_src, seq_dst, op_mac_src, op_mac_dst, mac_op_src, mac_op_dst = idxs
    feat_seq = tonp(feat_seq).astype(np.float32).ravel()
    feat_op_mac = tonp(feat_op_mac).astype(np.float32).ravel()
    feat_mac_op = tonp(feat_mac_op).astype(np.float32).ravel()
    W_op_w, W_op_b = tonp(W_op_w).astype(np.float32), tonp(W_op_b).astype(np.float32)
    W_mac_w, W_mac_b = tonp(W_mac_w).astype(np.float32), tonp(W_mac_b).astype(np.float32)
    att_seq, att_op_mac, att_mac_op = (tonp(att_seq).astype(np.float32),
                                       tonp(att_op_mac).astype(np.float32),
                                       tonp(att_mac_op).astype(np.float32))

    # ---- host param prep ----
    ps_seq, pd_seq, ac_seq = _proj(att_seq)
    ps_om, pd_om, ac_om = _proj(att_op_mac)
    ps_mo, pd_mo, ac_mo = _proj(att_mac_op)
    WT_op = W_op_w.T                      # [64,128]
    WT_mac = W_mac_w.T                    # [32,128]
    # s-table cols for op nodes: src_seq, dst_seq, src_om, dst_mo
    Q_op = np.concatenate([WT_op @ ps_seq, WT_op @ pd_seq,
                           WT_op @ ps_om, WT_op @ pd_mo], 1)      # [64,16]
    bq_op = np.concatenate([W_op_b @ ps_seq, W_op_b @ pd_seq,
                            W_op_b @ ps_om, W_op_b @ pd_mo])      # [16]
    # s-table cols for mac nodes: src_mo, dst_om (rest zero)
    Q_mac = np.concatenate([WT_mac @ ps_mo, WT_mac @ pd_om,
                            np.zeros((IN_MAC, 8), np.float32)], 1)
    bq_mac = np.concatenate([W_mac_b @ ps_mo, W_mac_b @ pd_om,
                             np.zeros(8, np.float32)])
    Wc_op = np.zeros((IN_OP + 1, 144), np.float32)
    Wc_op[:IN_OP, :OUT] = WT_op
    Wc_op[IN_OP, :OUT] = W_op_b
    Wc_op[:IN_OP, OUT:] = Q_op
    Wc_op[IN_OP, OUT:] = bq_op
    Wc_mac = np.zeros((IN_MAC + 1, 144), np.float32)
    Wc_mac[:IN_MAC, :OUT] = WT_mac
    Wc_mac[IN_MAC, :OUT] = W_mac_b
    Wc_mac[:IN_MAC, OUT:] = Q_mac
    Wc_mac[IN_MAC, OUT:] = bq_mac

    hT_op = np.zeros((IN_OP + 1, N_OP_PAD), np.float32)
    hT_op[:IN_OP, :N_OP] = h_op.T
    hT_op[IN_OP, :] = 1.0
    hT_mac = np.zeros((IN_MAC + 1, N_MAC_PAD), np.float32)
    hT_mac[:IN_MAC, :N_MAC] = h_mac.T
    hT_mac[IN_MAC, :] = 1.0

    consts_base = np.zeros((P, P + 12 + 4 * P), np.float32)
    consts_base[:, :P] = np.arange(P, dtype=np.float32)[None, :]
    consts_base[:, P:P + 4] = ac_seq[None, :]
    consts_base[:, P + 4:P + 8] = ac_mo[None, :]
    consts_base[:, P + 8:P + 12] = ac_om[None, :]
    consts_base[:, P + 12:P + 12 + P] = tonp(ln_op_g).astype(np.float32)[None, :]
    consts_base[:, P + 12 + P:P + 12 + 2 * P] = tonp(ln_op_b).astype(np.float32)[None, :]
    consts_base[:, P + 12 + 2 * P:P + 12 + 3 * P] = tonp(ln_mac_g).astype(np.float32)[None, :]
    consts_base[:, P + 12 + 3 * P:P + 12 + 4 * P] = tonp(ln_mac_b).astype(np.float32)[None, :]

    nblk_seq, pk_seq = _prep_edges(seq_src, seq_dst, feat_seq, ND_OP, T_OP)
    nblk_mo, pk_mo = _prep_edges(mac_op_src, mac_op_dst, feat_mac_op, ND_OP, T_OP)
    nblk_om, pk_om = _prep_edges(op_mac_src, op_mac_dst, feat_op_mac, ND_MAC, T_MAC)

    in_maps = []
    for c in range(NC):
        m = {"hT_op": hT_op, "hT_mac": hT_mac,
             "Wc_op": Wc_op, "Wc_mac": Wc_mac, "consts": consts_base}
        ho = np.zeros((IN_OP + 1, T_OP * P), np.float32)
        ho[:, :ND_OP] = hT_op[:, c * ND_OP:(c + 1) * ND_OP]
        m["hT_own_op"] = ho
        hm = np.zeros((IN_MAC + 1, T_MAC * P), np.float32)
        hm[:, :ND_MAC] = hT_mac[:, c * ND_MAC:(c + 1) * ND_MAC]
        m["hT_own_mac"] = hm
        for nm, pk in (("seq", pk_seq), ("mo", pk_mo), ("om", pk_om)):
            m[f"{nm}_zidx"] = pk[c]['zidx']
            m[f"{nm}_dloc"] = pk[c]['dloc']
            m[f"{nm}_drel"] = pk[c]['drel']
            m[f"{nm}_feat"] = pk[c]['feat']
        in_maps.append(m)

    nc = build_kernel(nblk_seq, nblk_mo, nblk_om)
    import os
    res = bass_utils.run_bass_kernel_spmd(
        nc, in_maps, core_ids=list(range(NC)),
        trace=bool(os.environ.get("BASS_TRACE")))
    _last_results[0] = res
    res_op = np.concatenate([res.results[c]["out_op"] for c in range(NC)], 0)
    res_mac = np.concatenate([res.results[c]["out_mac"] for c in range(NC)], 0)
    return res_op, res_mac
